# revision 5
# baseline (speedup 1.0000x reference)
"""Bilateral filter (d=7, sigma_color=0.1, sigma_space=3.0) on 8 Trainium2 cores.

Gaussian-sum (shiftable) decomposition with least-squares-fitted weights:
  exp(-50(s-x)^2) ~= sum_k c_k B_k(s) B_k(x),  B_k(u) = exp(-100(u-q_k)^2)
with q_k = linspace(0,1,K) and c_k solving  sum_k c_k exp(-200(m-q_k)^2) ~= 1
on m in [0,1] (the ripple H(m)-1 is the only approximation error).

Per (plane, col-band) unit on each core (BIN=128-wide bands -> every
data-as-lhsT matmul has 128 weight columns = fast weight load):
  - ACT Derivative_Erf gives B_k row-images, fp16
  - P_k = B_k * x16 (DVE/GPSIMD TT)
  - H-conv FUSED with transpose: matmul(lhsT=P_k piece [128r x 128c],
    rhs=Bband [128 x 114]) -> psum [128c x 114r'] (col-major immediately)
  - W-conv: matmul(lhsT=Cband [128 x 128], rhs=Y^T [128 x 768]) -> Z psum
  - modulate t_k = (c_k * B_k^T) * Z; B_k^T either recomputed by ACT from
    x^T (btk_pe=0) or PE-transposed from B_k (btk_pe=1); Z either read
    1x from psum via STT (tk_split=0) or evicted+scaled to SBUF first
    (tk_split=1, 2x TT)
  - accumulate via identity matmul into psum; transpose back via 6x128
    matmuls; DMA from SBUF to DRAM.
Rows are uniformly tiled: 7 tiles x 114 out rows, input padded to 812 rows
x 814 cols so every tile/band is full-size (single DMA, single evict copy).
"""
import json
import os
import numpy as np

D = 7
SIGMA_COLOR = 0.1
SIGMA_SPACE = 3.0

N_CORES = 8
PLANES = 6              # per-core planes (2 images x 3 channels)
H = W = 768
RPAD = 812              # padded rows: 7 + 768 + 37 (uniform 7x(114+14) tiles)
CPAD = 814              # padded cols: 7 + 768 + 39 (uniform 7x128 bands)
NB = 7                  # col bands
BW = 114                # out cols per band (7*114=798 >= 768)
BIN = 128               # in cols per band (= BW + 2*D = FWL-friendly 128)
NT = 7                  # row tiles per plane
TH = 114                # out rows per tile
THT = NT * TH           # 798
K_IMG = 10              # gaussian basis images
A_BASIS = 100.0
Q_MARGIN = 0.0

_CACHE = {}


def _grid(k_img=K_IMG, margin=Q_MARGIN):
    """LS fit: H(m) = sum_k c_k exp(-2A(m-q_k)^2) ~= 1 on [0,1].
    B_hw = DErf(10(u-q)) = 2/sqrt(pi) exp(-100(u-q)^2); per-k scalars carry
    c_k * pi/4 for the two DErf prefactors."""
    q = np.linspace(-margin, 1.0 + margin, k_img)
    m = np.linspace(0.0, 1.0, 4001)
    phi = np.exp(-2 * A_BASIS * (m[:, None] - q[None, :]) ** 2)
    c = np.linalg.solve(phi.T @ phi, phi.T @ np.ones(len(m)))
    wgts = [float(ck * np.pi / 4.0) for ck in c]
    return q, wgts


def _g1n():
    offs = np.arange(-D, D + 1)
    g = np.exp(-0.5 * offs ** 2 / SIGMA_SPACE ** 2)
    return (g / g.sum()).astype(np.float64)


def _sw00():
    g = np.exp(-0.5 * np.arange(-D, D + 1) ** 2 / SIGMA_SPACE ** 2)
    sw = np.outer(g, g)
    return float((sw / sw.sum())[D, D])


def _consts():
    g1n = _g1n()
    bband = np.zeros((128, TH), np.float16)
    for ri in range(128):
        for ro in range(TH):
            d = ri - ro
            if 0 <= d <= 2 * D:
                bband[ri, ro] = g1n[d]
    # cband maps in-col ci (0..127) to out partition p=co+7 (7..120)
    cband = np.zeros((BIN, BIN), np.float16)
    for ci in range(BIN):
        for p in range(BIN):
            dd = ci - (p - D)
            if D <= p < D + BW and 0 <= dd <= 2 * D:
                cband[ci, p] = g1n[dd]
    ident16 = np.eye(128, dtype=np.float16)
    idneg = (-_sw00() * np.eye(BIN)).astype(np.float16)
    return bband, cband, ident16, idneg


DEFAULTS = dict(
    k_img=K_IMG,
    btk_pe=0,        # 1: B_k^T via PE transpose of B_k (kills 2nd ACT eval)
    tk_split=0,      # 1: evict Z*c_k to SBUF (zk) then 2x TT, vs 1x STT
    zk_act=10,       # of k's whose zk evict goes to ACT (rest DVE)
    evict_dve=5,     # of k's whose Y-evict goes to DVE (rest ACT)
    bt_dve=10,       # of k's whose B_k^T evict goes to DVE (rest ACT)
    p_gpsimd=0,      # of P-products routed to GPSIMD
    copies_gp=True,  # x16 conversion on GPSIMD
    sb_bufs=7, ph_bufs=2, pz_bufs=1, pa_bufs=1,
    xt_dve=True,
)


def build(reps=1, **overrides):
    cfg = dict(DEFAULTS)
    cfg.update(json.loads(os.environ.get("KERNEL_BUILD_KWARGS", "{}")))
    cfg.update(overrides)
    k_img = cfg["k_img"]
    btk_pe, tk_split = cfg["btk_pe"], cfg["tk_split"]
    zk_act, evict_dve, bt_dve = cfg["zk_act"], cfg["evict_dve"], cfg["bt_dve"]
    p_gpsimd, copies_gp = cfg["p_gpsimd"], cfg["copies_gp"]
    sb_bufs, ph_bufs = cfg["sb_bufs"], cfg["ph_bufs"]
    pz_bufs, pa_bufs = cfg["pz_bufs"], cfg["pa_bufs"]
    xt_dve = cfg["xt_dve"]

    import concourse.tile as tile
    import concourse.bass as bass
    from concourse import bacc, mybir

    f32 = mybir.dt.float32
    fp16 = mybir.dt.float16
    AF = mybir.ActivationFunctionType
    ALU = mybir.AluOpType

    q, wgts = _grid(k_img)
    nc = bacc.Bacc("TRN2", target_bir_lowering=False, debug=False,
                   num_devices=N_CORES)
    xp = nc.dram_tensor("xp", [PLANES, RPAD, CPAD], f32, kind="ExternalInput")
    out = nc.dram_tensor("out", [PLANES, H, W], f32, kind="ExternalOutput")
    bband_d = nc.dram_tensor("bband", [128, TH], fp16, kind="ExternalInput")
    cband_d = nc.dram_tensor("cband", [BIN, BIN], fp16, kind="ExternalInput")
    ident16_d = nc.dram_tensor("ident16", [128, 128], fp16, kind="ExternalInput")
    idneg_d = nc.dram_tensor("idneg", [BIN, BIN], fp16, kind="ExternalInput")

    with tile.TileContext(nc) as tc:
        with (
            tc.tile_pool(name="consts", bufs=1) as consts,
            tc.tile_pool(name="xf_pool", bufs=2) as xf_pool,
            tc.tile_pool(name="x16_pool", bufs=2) as x16_pool,
            tc.tile_pool(name="xt_pool", bufs=2) as xt_pool,
            tc.tile_pool(name="b_pool", bufs=sb_bufs) as b_pool,
            tc.tile_pool(name="p_pool", bufs=sb_bufs) as p_pool,
            tc.tile_pool(name="y_pool", bufs=sb_bufs) as y_pool,
            tc.tile_pool(name="bt_pool", bufs=sb_bufs) as bt_pool,
            tc.tile_pool(name="z_pool", bufs=sb_bufs) as z_pool,
            tc.tile_pool(name="t_pool", bufs=sb_bufs) as t_pool,
            tc.tile_pool(name="of_pool", bufs=2) as of_pool,
            tc.tile_pool(name="ph_pool", bufs=ph_bufs, space="PSUM") as ph_pool,
            tc.tile_pool(name="pz_pool", bufs=pz_bufs, space="PSUM") as pz_pool,
            tc.tile_pool(name="pa_pool", bufs=pa_bufs, space="PSUM") as pa_pool,
        ):
            bb = consts.tile([128, TH], fp16)
            nc.sync.dma_start(out=bb[:], in_=bband_d.ap())
            cb = consts.tile([BIN, BIN], fp16)
            nc.sync.dma_start(out=cb[:], in_=cband_d.ap())
            id16 = consts.tile([128, 128], fp16)
            nc.sync.dma_start(out=id16[:], in_=ident16_d.ap())
            idn = consts.tile([BIN, BIN], fp16)
            nc.sync.dma_start(out=idn[:], in_=idneg_d.ap())
            biases = consts.tile([128, k_img], f32)
            for k in range(k_img):
                nc.gpsimd.memset(biases[:, k:k + 1], float(-10.0 * q[k]))

            def unit(plane, band):
                c0 = BW * band            # first out col
                ncols = min(BW, W - c0)   # 114, or 84 for last band
                # ---- load x band [128, 7, 128] f32 (uniform row tiles)
                xf = xf_pool.tile([128, NT, BIN], f32, tag="xf")
                base = plane * RPAD * CPAD + c0
                nc.sync.dma_start(
                    out=xf[:],
                    in_=bass.AP(tensor=xp, offset=base,
                                ap=[[CPAD, 128], [TH * CPAD, NT], [1, BIN]]))
                x16 = x16_pool.tile([128, NT, BIN], fp16, tag="x16")
                (nc.gpsimd if copies_gp else nc.vector).tensor_copy(x16[:], xf[:])
                # ---- x^T via matmul transpose (fp16 lanes; values exact)
                pxt = ph_pool.tile([BIN, NT, 128], f32, tag="ph")
                for t in range(NT):
                    nc.tensor.matmul(pxt[:, t, 0:128], x16[:, t, :], id16[:],
                                     start=True, stop=True)
                xt16 = xt_pool.tile([BIN, THT], fp16, tag="xt16")
                xte = nc.vector if xt_dve else nc.scalar
                if xt_dve:
                    nc.vector.tensor_copy(xt16[:], pxt[:, :, D:D + TH])
                else:
                    nc.scalar.copy(xt16[:], pxt[:, :, D:D + TH])

                pacc = pa_pool.tile([BIN, H], f32, tag="pa")
                nmm = [0]
                LAST = 2 * (k_img + 1)

                def acc_mm(lhsT, rhs):
                    for (a, b) in ((0, 512), (512, H)):
                        nc.tensor.matmul(pacc[:, a:b], lhsT, rhs[:, a:b],
                                         start=(nmm[0] < 2),
                                         stop=(nmm[0] >= LAST - 2))
                        nmm[0] += 1

                for k in range(k_img):
                    bk = b_pool.tile([128, NT, BIN], fp16, tag="bk")
                    nc.scalar.activation(bk[:], xf[:], AF.Derivative_Erf,
                                         scale=10.0, bias=biases[:, k:k + 1])
                    pk = p_pool.tile([128, NT, BIN], fp16, tag="pk")
                    peng = nc.gpsimd if (k % k_img) < p_gpsimd else nc.vector
                    peng.tensor_tensor(pk[:], bk[:], x16[:], ALU.mult)
                    # H-conv + transpose fused (lhsT 128 cols -> FWL)
                    ph = ph_pool.tile([BIN, NT, 128], f32, tag="ph")
                    for t in range(NT):
                        nc.tensor.matmul(ph[:, t, 0:TH], pk[:, t, :], bb[:],
                                         start=True, stop=True)
                    yk = y_pool.tile([BIN, THT], fp16, tag="yk")
                    yeng = nc.vector if (k % k_img) < evict_dve else nc.scalar
                    if yeng is nc.vector:
                        nc.vector.tensor_copy(yk[:], ph[:, :, 0:TH])
                    else:
                        nc.scalar.copy(yk[:], ph[:, :, 0:TH])
                    # W-conv
                    pz = pz_pool.tile([BIN, H], f32, tag="pz")
                    nc.tensor.matmul(pz[:, 0:512], cb[:], yk[:, 0:512],
                                     start=True, stop=True)
                    nc.tensor.matmul(pz[:, 512:H], cb[:], yk[:, 512:H],
                                     start=True, stop=True)
                    # B_k^T: either PE-transpose of bk, or ACT from x^T
                    btk = bt_pool.tile([BIN, THT], fp16, tag="btk")
                    if btk_pe:
                        pbt = ph_pool.tile([BIN, NT, 128], f32, tag="ph")
                        for t in range(NT):
                            nc.tensor.matmul(pbt[:, t, 0:128], bk[:, t, :],
                                             id16[:], start=True, stop=True)
                        bteng = nc.vector if (k % k_img) < bt_dve else nc.scalar
                        if bteng is nc.vector:
                            nc.vector.tensor_copy(btk[:], pbt[:, :, D:D + TH])
                        else:
                            nc.scalar.copy(btk[:], pbt[:, :, D:D + TH])
                    else:
                        nc.scalar.activation(btk[:, 0:H], xt16[:, 0:H],
                                             AF.Derivative_Erf, scale=10.0,
                                             bias=biases[0:BIN, k:k + 1])
                    tk = t_pool.tile([BIN, H], fp16, tag="tk")
                    if tk_split:
                        zk = z_pool.tile([BIN, H], fp16, tag="zk")
                        if (k % k_img) < zk_act:
                            nc.scalar.mul(zk[:], pz[:], wgts[k])
                        else:
                            nc.vector.tensor_scalar_mul(zk[:], pz[:], wgts[k])
                        nc.vector.tensor_tensor(tk[:], btk[:, 0:H], zk[:],
                                                ALU.mult)
                    else:
                        nc.vector.scalar_tensor_tensor(tk[:], btk[:, 0:H],
                                                       wgts[k], pz[:],
                                                       ALU.mult, ALU.mult)
                    acc_mm(id16[0:BIN, 0:BIN], tk)
                # center term: -sw00 * x^T
                acc_mm(idn[:], xt16[:, 0:H])
                # evict accumulator, transpose back (6x128 rows), DMA out
                of = of_pool.tile([BIN, H], fp16, tag="of")
                nc.vector.tensor_copy(of[:], pacc[:])
                pb = ph_pool.tile([BIN, NT, 128], f32, tag="ph")
                for t in range(6):
                    nc.tensor.matmul(pb[0:128, t, 0:ncols],
                                     of[:, 128 * t:128 * (t + 1)],
                                     id16[0:BIN, D:D + ncols],
                                     start=True, stop=True)
                ob = of_pool.tile([128, 6, BW], f32, tag="ob")
                oeng = nc.scalar if (plane + band) % 2 == 0 else nc.vector
                if oeng is nc.scalar:
                    nc.scalar.copy(ob[:, :, 0:ncols], pb[:, 0:6, 0:ncols])
                else:
                    nc.vector.tensor_copy(ob[:, :, 0:ncols], pb[:, 0:6, 0:ncols])
                obase = plane * H * W + c0
                nc.sync.dma_start(
                    out=bass.AP(tensor=out, offset=obase,
                                ap=[[W, 128], [128 * W, 6], [1, ncols]]),
                    in_=ob[:, :, 0:ncols])

            def body(_iv=None):
                for plane in range(PLANES):
                    for band in range(NB):
                        unit(plane, band)

            if reps == 1:
                body()
            else:
                with tc.For_i(0, reps, 1) as _i:
                    body(_i)
    nc.compile()
    return nc


def _prepare_inputs(x):
    """x: [16,3,768,768] f32 -> per-core padded plane stacks + consts."""
    planes = np.ascontiguousarray(x.reshape(N_CORES, PLANES, H, W))
    bband, cband, ident16, idneg = _consts()
    in_maps = []
    for c in range(N_CORES):
        xpad = np.pad(planes[c], ((0, 0), (D, RPAD - H - D), (D, CPAD - W - D)),
                      mode="reflect")
        in_maps.append({"xp": np.ascontiguousarray(xpad),
                        "bband": bband, "cband": cband, "ident16": ident16,
                        "idneg": idneg})
    return in_maps


def _gather_outputs(results):
    outs = [results[c]["out"] for c in range(N_CORES)]
    return np.stack(outs).reshape(16, 3, H, W).astype(np.float32)


def kernel(x):
    from concourse.bass_utils import run_bass_kernel_spmd

    x = np.asarray(x, dtype=np.float32)
    if "nc" not in _CACHE:
        _CACHE["nc"] = build(reps=1)
    in_maps = _prepare_inputs(x)
    res = run_bass_kernel_spmd(_CACHE["nc"], in_maps,
                               core_ids=list(range(N_CORES)))
    return _gather_outputs(res.results)
